# revision 24
# baseline (speedup 1.0000x reference)
"""HiRA layer (rank-modulated linear) Trainium2 kernel.

Computes out = x @ (W * (1 + A^T B^T)^T)^T + bias for
x:[4,2048,4096] f32, W:[4096,4096], A:[16,4096], B:[4096,16], bias:[4096].

Sharding: 2-way over tokens x 4-way over out-features (8 NeuronCores).
Each core:
  1. builds its adapted-weight shard on device:
     P'[i,o] = sum_r A_aug[r,i] * B_aug^T[r,o]   (ones-row augmentation
     folds the +1 into the matmul; pairs of modulation matmuls run
     CONCURRENTLY in different PE row groups via tile_position, using a
     partition-replicated copy of A_aug/B_aug^T), then
     AWT[i,o] = W^T[i,o] * P'[i,o] on DVE, cast bf16, resident in SBUF.
  2. streams x tiles (host pre-blocked to [m, p=i, k, t=tok] bf16) through
     the PE: psum[tok,o] = sum_k XB[m,:,k,:].T @ AWT[k-chunk, o-slice],
     adds bias on DVE during the PSUM->SBUF copy, DMAs out f32.

Schedule: single pass over x.  Phase B produces og0 chunks k-outer,
overlapped with main matmuls on the first M0 token tiles (staggered
group starts matched to DMA arrival); og1 chunk production starts in
B's tail and finishes during phase C (og1 mains for the same tiles);
phase D streams the remaining tiles at the N=512 matmul roofline.

Host side only reshapes/transposes/casts and slices shards; every FLOP of
the reference computation happens on device.
"""

import sys

for _p in ("/opt/trn_rl_repo",):
    if _p not in sys.path:
        sys.path.insert(0, _p)

import numpy as np
import ml_dtypes

BF16 = ml_dtypes.bfloat16

# problem shape (hardcoded per contract)
B, S, IN, OUT, R = 4, 2048, 4096, 4096, 16
TOK = B * S            # 8192
TB, OB = 2, 4          # token-halves x out-feature quarters = 8 cores
TOKH = TOK // TB       # 4096 tokens per core
OQ = OUT // OB         # 1024 out features per core
MT = TOKH // 128       # 32 token tiles
KT = IN // 128         # 32 contraction chunks
N_CORES = 8
RP = 49                # replicated lora rows: [0:17] and [32:49]

M0 = 4                 # token tiles processed during chunk production
BSTART = (0, 12, 20, 26)   # phase-B group start chunks (even)
CS = (0, 2, 4, 6)          # phase-C group start offsets (even)
SCRATCH = 28           # HAM warm-up matmuls bridging DMA cold-start

TRACE = False          # test.py sets True to capture NTFF exec time
LAST_RESULT = None     # BassKernelResults of the most recent run

_NC_CACHE = None


def _build_nc():
    import concourse.bass as bass
    import concourse.bacc as bacc
    import concourse.mybir as mybir
    from concourse import tile

    f32 = mybir.dt.float32
    bf16 = mybir.dt.bfloat16

    nc = bacc.Bacc(
        "TRN2", target_bir_lowering=False, debug=False, num_devices=N_CORES
    )

    XB = nc.dram_tensor("xb", [MT, 128, KT, 128], bf16, kind="ExternalInput")
    WT0 = nc.dram_tensor("wt0", [KT // 2, 128, 2, 512], bf16, kind="ExternalInput")
    WT1 = nc.dram_tensor("wt1", [KT // 2, 128, 2, 512], bf16, kind="ExternalInput")
    AAUG = nc.dram_tensor("a_aug", [RP, IN], bf16, kind="ExternalInput")
    BTAUG = nc.dram_tensor("bt_aug", [RP, OQ], bf16, kind="ExternalInput")
    BIASB = nc.dram_tensor("bias_b", [128, OQ], f32, kind="ExternalInput")
    OUTP = nc.dram_tensor("out", [MT, 128, OQ], f32, kind="ExternalOutput")

    WTH = (WT0, WT1)

    # ---- build-time schedule checker (host-side only) ----
    emitted = {"n": 0}
    mul_done = {}      # (og, k) -> emission idx of its DVE mul
    pp_hist = []       # ppp alloc order: (og, k) or ("scr", i)
    po_hist = []       # opp alloc order: group label
    po_freed = {}      # group label -> emission idx of its drain

    def tick():
        emitted["n"] += 1
        return emitted["n"]

    def pp_alloc(label):
        if len(pp_hist) >= 2:
            old = pp_hist[-2]
            assert old[0] == "scr" or old in mul_done, (
                f"pp slot reuse for {label} before reader of {old} emitted"
            )
        pp_hist.append(label)

    def po_alloc(label):
        if len(po_hist) >= 4:
            old = po_hist[-4]
            assert old in po_freed, (
                f"opp slot reuse for {label} before drain of {old}"
            )
        po_hist.append(label)

    with tile.TileContext(nc) as tc:
        with (
            tc.tile_pool(name="const", bufs=1) as const,
            tc.tile_pool(name="awt", bufs=1) as awtp,
            tc.tile_pool(name="wtld", bufs=12) as wtld,
            tc.tile_pool(name="xb", bufs=8) as xbp,
            tc.tile_pool(name="ob", bufs=4) as obp,
            tc.tile_pool(name="pps", bufs=2, space=bass.MemorySpace.PSUM) as ppp,
            tc.tile_pool(name="opsum", bufs=4, space=bass.MemorySpace.PSUM) as opp,
        ):
            a_t = const.tile([RP, IN], bf16)
            bt_t = const.tile([RP, OQ], bf16)
            bias_t = const.tile([128, OQ], f32)
            wu_l = const.tile([128, 128], bf16)
            wu_r = const.tile([128, 512], bf16)

            # adapted weight, bf16, resident: [p=i%128, k=i//128, o]
            awt = awtp.tile([128, KT, OQ], bf16)

            xbt = {}
            pend = {}
            mods = {}

            def load_xb(m):
                xbt[m] = xbp.tile([128, KT, 128], bf16, tag="xb", name=f"xbt{m}")
                nc.sync.dma_start(out=xbt[m][:], in_=XB[m])

            def wt_dma(og, p):
                """Stage the og-half of chunk pair p (chunks 2p, 2p+1)."""
                t = wtld.tile([128, 2, 512], bf16, tag="wt")
                nc.sync.dma_start(out=t[:], in_=WTH[og][p])
                pend[(og, p)] = t

            def mod_pair(og, p):
                """Two K=17 modulation matmuls for chunk pair p run
                concurrently in row groups q0 / q1, into one 2-bank tile."""
                k = 2 * p
                osl = slice(og * 512, (og + 1) * 512)
                pp = ppp.tile([128, 2, 512], f32, tag="pp", name=f"pp{og}_{p}")
                pp_alloc((og, p))
                nc.tensor.matmul(
                    pp[:, 0, :],
                    a_t[0:R + 1, k * 128:(k + 1) * 128],
                    bt_t[0:R + 1, osl],
                    start=True, stop=True, tile_position=(0, 0),
                )
                nc.tensor.matmul(
                    pp[:, 1, :],
                    a_t[32:32 + R + 1, (k + 1) * 128:(k + 2) * 128],
                    bt_t[32:32 + R + 1, osl],
                    start=True, stop=True, tile_position=(32, 0),
                )
                mods[(og, p)] = pp

            def mul_pair(og, p):
                """One wide DVE multiply covering both chunks of pair p."""
                k = 2 * p
                osl = slice(og * 512, (og + 1) * 512)
                nc.vector.tensor_mul(
                    awt[:, k:k + 2, osl],
                    mods.pop((og, p))[:],
                    pend.pop((og, p))[:],
                )
                mul_done[(og, p)] = tick()
                mul_done[(og, k)] = mul_done[(og, k + 1)] = tick()

            def main_mm(po, m, og, k):
                assert (og, k) in mul_done, f"main ({m},{og},{k}) before mul"
                osl = slice(og * 512, (og + 1) * 512)
                nc.tensor.matmul(
                    po[:],
                    xbt[m][:, k, :],
                    awt[:, k, osl],
                    start=(k == 0),
                    stop=(k == KT - 1),
                )

            def drain(po, label, m, og):
                osl = slice(og * 512, (og + 1) * 512)
                ot = obp.tile([128, 512], f32, tag="ot")
                nc.vector.tensor_add(ot[:], po[:], bias_t[:, osl])
                nc.sync.dma_start(out=OUTP[m, :, osl], in_=ot[:])
                po_freed[label] = tick()

            # ---- prologue ----
            nc.sync.dma_start(out=a_t[:], in_=AAUG[:])
            nc.sync.dma_start(out=bt_t[:], in_=BTAUG[:])
            for p in range(2):
                wt_dma(0, p)
            load_xb(0)
            for p in range(2, 4):
                wt_dma(0, p)
            load_xb(1)

            nc.vector.memset(wu_l[:], 0.0)
            nc.vector.memset(wu_r[:], 0.0)
            for i in range(SCRATCH):
                pp = ppp.tile([128, 2, 512], f32, tag="pp", name=f"scr{i}")
                pp_alloc(("scr", i))
                nc.tensor.matmul(
                    pp[:, 0, :], wu_l[:], wu_r[:], start=True, stop=True
                )

            mod_pair(0, 0)

            # ---- phase B: og0 production + staggered og0 mains m=0..3;
            #      og1 production begins in the tail ----
            po0 = []
            for m in range(M0):
                po_alloc(("og0", m))
                po0.append(
                    opp.tile([128, 512], f32, tag="po", name=f"po0_{m}")
                )
            NJ_B = BSTART[-1] // 2 + KT // 2 + 1  # 30
            for j in range(NJ_B):
                if j == 0:
                    load_xb(2)
                if j == 1:
                    load_xb(3)
                if j == 2:
                    nc.sync.dma_start(out=bias_t[:], in_=BIASB[:])
                # og0 weight staging, 4 pairs ahead
                if j + 4 < KT // 2:
                    wt_dma(0, j + 4)
                # og0 mul for pair j
                if j < KT // 2:
                    mul_pair(0, j)
                # og0 mods one pair ahead
                if j + 1 < KT // 2:
                    mod_pair(0, j + 1)
                # PE densifier while only group 0 is active
                if j <= 4:
                    nc.tensor.matmul(
                        po0[3][:], wu_l[:], wu_r[:],
                        start=True, stop=True, skip_group_check=True,
                    )
                # ---- og1 production dripped into the tail ----
                if j == 15:
                    wt_dma(1, 0)
                if j == 16:
                    wt_dma(1, 1)
                    mod_pair(1, 0)
                if j == 17:
                    mul_pair(1, 0)
                    mod_pair(1, 1)
                if j == 18:
                    wt_dma(1, 2)
                    mul_pair(1, 1)
                    mod_pair(1, 2)
                # og0 mains, block j-1
                for m in range(M0):
                    for t in (0, 1):
                        k = 2 * (j - 1) + t - BSTART[m]
                        if 0 <= k < KT:
                            main_mm(po0[m], m, 0, k)
                # og0-m0 drain right after its stop (j == 17)
                if j == 17:
                    drain(po0[0], ("og0", 0), 0, 0)

            # ---- phase C: og1 mains m=0..3 + remaining og1 production ----
            po1 = {}
            for j in range(CS[-1] // 2 + KT // 2):  # 19
                if j == 4:
                    load_xb(4)
                if j == 10:
                    load_xb(5)
                # og1 weight staging, pairs ahead
                if j + 3 < KT // 2:
                    wt_dma(1, j + 3)
                # og1 mul one pair ahead of mains
                if j + 2 < KT // 2:
                    mul_pair(1, j + 2)
                # og1 mods two pairs ahead
                if j + 3 < KT // 2:
                    mod_pair(1, j + 3)
                # drains of og0 groups, one iter before og1 group m starts
                m = j + 1
                if 1 <= m < M0:
                    drain(po0[m], ("og0", m), m, 0)
                # og1 group starts: group m starts mains at j == CS[m] // 2
                for m in range(M0):
                    if j == CS[m] // 2:
                        po_alloc(("og1", m))
                        po1[m] = opp.tile(
                            [128, 512], f32, tag="po", name=f"po1_{m}"
                        )
                # og1 mains, block j
                for m in range(M0):
                    for t in (0, 1):
                        k = 2 * j + t - CS[m]
                        if 0 <= k < KT:
                            main_mm(po1[m], m, 1, k)

            # ---- phase D: m=4..31, both output halves per x tile ----
            for m in range(M0):
                drain(po1[m], ("og1", m), m, 1)
            for m in range(M0, MT):
                if m + 2 < MT:
                    load_xb(m + 2)
                for og in range(2):
                    if m == MT - 1 and og == 1:
                        # final group: two sequential 256-wide halves in
                        # separate full banks; half A's drain hides under
                        # half B's accumulation, shrinking the exposed tail
                        for h, (o0, o1) in enumerate(((512, 768), (768, 1024))):
                            po_alloc(("d", m, og, h))
                            po_freed[("d", m, og, h)] = tick()
                            ph = opp.tile(
                                [128, 512], f32, tag="po", name=f"ph{h}"
                            )
                            for k in range(KT):
                                nc.tensor.matmul(
                                    ph[:, 0:256],
                                    xbt[m][:, k, :],
                                    awt[:, k, o0:o1],
                                    start=(k == 0),
                                    stop=(k == KT - 1),
                                )
                            ot = obp.tile(
                                [128, 256], f32, tag="ot", name=f"oth{h}"
                            )
                            nc.vector.tensor_add(
                                ot[:], ph[:, 0:256], bias_t[:, o0:o1]
                            )
                            nc.sync.dma_start(
                                out=OUTP[m, :, o0:o1], in_=ot[:]
                            )
                        continue
                    po_alloc(("d", m, og))
                    po = opp.tile([128, 512], f32, tag="po")
                    for k in range(KT):
                        main_mm(po, m, og, k)
                    drain(po, ("d", m, og), m, og)

            assert not mods and not pend, (list(mods), list(pend))

    nc.compile()
    return nc


def _get_nc():
    global _NC_CACHE
    if _NC_CACHE is None:
        _NC_CACHE = _build_nc()
    return _NC_CACHE


def kernel(x, weight, bias, lora_A, lora_B):
    global LAST_RESULT
    from concourse.bass_utils import run_bass_kernel_spmd

    x = np.asarray(x, dtype=np.float32)
    weight = np.asarray(weight, dtype=np.float32)
    bias = np.asarray(bias, dtype=np.float32)
    lora_A = np.asarray(lora_A, dtype=np.float32)
    lora_B = np.asarray(lora_B, dtype=np.float32)

    x2 = x.reshape(TOK, IN)

    # x blocked per token-half: [m, p=i%128, k=i//128, t=tok%128] bf16
    xbs = []
    for tb in range(TB):
        xh = x2[tb * TOKH:(tb + 1) * TOKH]
        xb = xh.reshape(MT, 128, KT, 128).transpose(0, 3, 2, 1)  # [m,p,k,t]
        xbs.append(np.ascontiguousarray(xb.astype(BF16)))

    # lora consts replicated at partition offsets 0 and 32 (row-group pairs)
    a1 = np.concatenate([lora_A, np.ones((1, IN), np.float32)], axis=0)
    a_aug = np.zeros((RP, IN), np.float32)
    a_aug[0:R + 1] = a1
    a_aug[32:32 + R + 1] = a1
    a_aug = a_aug.astype(BF16)

    wt0s, wt1s, bts, biases = [], [], [], []
    for ob in range(OB):
        osl = slice(ob * OQ, (ob + 1) * OQ)
        wq = weight[osl]                                   # [OQ, IN]
        wt = wq.T.reshape(KT, 128, OQ).astype(BF16)        # [k, p, o] bf16
        # paired: [pair, p, t(chunk-in-pair), o-half]
        wt0s.append(np.ascontiguousarray(
            wt[:, :, 0:512].reshape(KT // 2, 2, 128, 512).transpose(0, 2, 1, 3)
        ))
        wt1s.append(np.ascontiguousarray(
            wt[:, :, 512:1024].reshape(KT // 2, 2, 128, 512).transpose(0, 2, 1, 3)
        ))
        bq = lora_B[osl]                                   # [OQ, R]
        b1 = np.concatenate([bq.T, np.ones((1, OQ), np.float32)], axis=0)
        bt = np.zeros((RP, OQ), np.float32)
        bt[0:R + 1] = b1
        bt[32:32 + R + 1] = b1
        bts.append(bt.astype(BF16))
        biases.append(np.ascontiguousarray(np.tile(bias[osl][None, :], (128, 1))))

    in_maps = []
    for c in range(N_CORES):
        tb, ob = c // OB, c % OB
        in_maps.append(
            {
                "xb": xbs[tb],
                "wt0": wt0s[ob],
                "wt1": wt1s[ob],
                "a_aug": a_aug,
                "bt_aug": bts[ob],
                "bias_b": biases[ob],
            }
        )

    nc = _get_nc()
    res = run_bass_kernel_spmd(
        nc, in_maps, core_ids=list(range(N_CORES)), trace=TRACE
    )
    LAST_RESULT = res

    # reassemble: out[c] is [MT, 128, OQ] -> [TOKH, OQ]
    halves = []
    for tb in range(TB):
        cols = [
            res.results[tb * OB + ob]["out"].reshape(TOKH, OQ)
            for ob in range(OB)
        ]
        halves.append(np.concatenate(cols, axis=1))
    full = np.concatenate(halves, axis=0).reshape(B, S, OUT)
    return full


# revision 25
# speedup vs baseline: 1.0970x; 1.0970x over previous
"""HiRA layer (rank-modulated linear) Trainium2 kernel.

Computes out = x @ (W * (1 + A^T B^T)^T)^T + bias for
x:[4,2048,4096] f32, W:[4096,4096], A:[16,4096], B:[4096,16], bias:[4096].

Sharding: 2-way over tokens x 4-way over out-features (8 NeuronCores).
Each core:
  1. builds its adapted-weight shard on device:
     P'[i,o] = sum_r A_aug[r,i] * B_aug^T[r,o]   (ones-row augmentation
     folds the +1 into the matmul; pairs of modulation matmuls run
     CONCURRENTLY in different PE row groups via tile_position, using a
     partition-replicated copy of A_aug/B_aug^T), then
     AWT[i,o] = W^T[i,o] * P'[i,o] on DVE, cast bf16, resident in SBUF.
  2. streams x tiles (host pre-blocked to [m, p=i, k, t=tok] bf16) through
     the PE: psum[tok,o] = sum_k XB[m,:,k,:].T @ AWT[k-chunk, o-slice],
     adds bias on DVE during the PSUM->SBUF copy, DMAs out f32.

Schedule: single pass over x.  Phase B produces og0 chunks k-outer,
overlapped with main matmuls on the first M0 token tiles (staggered
group starts matched to DMA arrival); og1 chunk production starts in
B's tail and finishes during phase C (og1 mains for the same tiles);
phase D streams the remaining tiles at the N=512 matmul roofline.

Host side only reshapes/transposes/casts and slices shards; every FLOP of
the reference computation happens on device.
"""

import sys

for _p in ("/opt/trn_rl_repo",):
    if _p not in sys.path:
        sys.path.insert(0, _p)

import numpy as np
import ml_dtypes

BF16 = ml_dtypes.bfloat16

# problem shape (hardcoded per contract)
B, S, IN, OUT, R = 4, 2048, 4096, 4096, 16
TOK = B * S            # 8192
TB, OB = 2, 4          # token-halves x out-feature quarters = 8 cores
TOKH = TOK // TB       # 4096 tokens per core
OQ = OUT // OB         # 1024 out features per core
MT = TOKH // 128       # 32 token tiles
KT = IN // 128         # 32 contraction chunks
N_CORES = 8
RP = 49                # replicated lora rows: [0:17] and [32:49]

M0 = 4                 # token tiles processed during chunk production
BSTART = (0, 12, 18, 26)   # phase-B group start chunks (even)
CS = (0, 2, 4, 6)          # phase-C group start offsets (even)
SCRATCH = 28           # HAM warm-up matmuls bridging DMA cold-start

TRACE = False          # test.py sets True to capture NTFF exec time
LAST_RESULT = None     # BassKernelResults of the most recent run

_NC_CACHE = None


def _build_nc():
    import concourse.bass as bass
    import concourse.bacc as bacc
    import concourse.mybir as mybir
    from concourse import tile

    f32 = mybir.dt.float32
    bf16 = mybir.dt.bfloat16

    nc = bacc.Bacc(
        "TRN2", target_bir_lowering=False, debug=False, num_devices=N_CORES
    )

    XB = nc.dram_tensor("xb", [MT, 128, KT, 128], bf16, kind="ExternalInput")
    WT0 = nc.dram_tensor("wt0", [KT // 2, 128, 2, 512], bf16, kind="ExternalInput")
    WT1 = nc.dram_tensor("wt1", [KT // 2, 128, 2, 512], bf16, kind="ExternalInput")
    AAUG = nc.dram_tensor("a_aug", [RP, IN], bf16, kind="ExternalInput")
    BTAUG = nc.dram_tensor("bt_aug", [RP, OQ], bf16, kind="ExternalInput")
    BIASB = nc.dram_tensor("bias_b", [128, OQ], f32, kind="ExternalInput")
    OUTP = nc.dram_tensor("out", [MT, 128, OQ], f32, kind="ExternalOutput")

    WTH = (WT0, WT1)

    # ---- build-time schedule checker (host-side only) ----
    emitted = {"n": 0}
    mul_done = {}      # (og, k) -> emission idx of its DVE mul
    pp_hist = []       # ppp alloc order: (og, k) or ("scr", i)
    po_hist = []       # opp alloc order: group label
    po_freed = {}      # group label -> emission idx of its drain

    def tick():
        emitted["n"] += 1
        return emitted["n"]

    def pp_alloc(label):
        if len(pp_hist) >= 2:
            old = pp_hist[-2]
            assert old[0] == "scr" or old in mul_done, (
                f"pp slot reuse for {label} before reader of {old} emitted"
            )
        pp_hist.append(label)

    def po_alloc(label):
        if len(po_hist) >= 4:
            old = po_hist[-4]
            assert old in po_freed, (
                f"opp slot reuse for {label} before drain of {old}"
            )
        po_hist.append(label)

    with tile.TileContext(nc) as tc:
        with (
            tc.tile_pool(name="const", bufs=1) as const,
            tc.tile_pool(name="awt", bufs=1) as awtp,
            tc.tile_pool(name="wtld", bufs=12) as wtld,
            tc.tile_pool(name="xb", bufs=8) as xbp,
            tc.tile_pool(name="ob", bufs=4) as obp,
            tc.tile_pool(name="pps", bufs=2, space=bass.MemorySpace.PSUM) as ppp,
            tc.tile_pool(name="opsum", bufs=4, space=bass.MemorySpace.PSUM) as opp,
        ):
            a_t = const.tile([RP, IN], bf16)
            bt_t = const.tile([RP, OQ], bf16)
            bias_t = const.tile([128, OQ], f32)
            wu_l = const.tile([128, 128], bf16)
            wu_r = const.tile([128, 512], bf16)

            # adapted weight, bf16, resident: [p=i%128, k=i//128, o]
            awt = awtp.tile([128, KT, OQ], bf16)

            xbt = {}
            pend = {}
            mods = {}

            def load_xb(m):
                xbt[m] = xbp.tile([128, KT, 128], bf16, tag="xb", name=f"xbt{m}")
                nc.sync.dma_start(out=xbt[m][:], in_=XB[m])

            def wt_dma(og, p):
                """Stage the og-half of chunk pair p (chunks 2p, 2p+1)."""
                t = wtld.tile([128, 2, 512], bf16, tag="wt")
                nc.sync.dma_start(out=t[:], in_=WTH[og][p])
                pend[(og, p)] = t

            def mod_pair(og, p):
                """Two K=17 modulation matmuls for chunk pair p run
                concurrently in row groups q0 / q1, into one 2-bank tile."""
                k = 2 * p
                osl = slice(og * 512, (og + 1) * 512)
                pp = ppp.tile([128, 2, 512], f32, tag="pp", name=f"pp{og}_{p}")
                pp_alloc((og, p))
                nc.tensor.matmul(
                    pp[:, 0, :],
                    a_t[0:R + 1, k * 128:(k + 1) * 128],
                    bt_t[0:R + 1, osl],
                    start=True, stop=True, tile_position=(0, 0),
                )
                nc.tensor.matmul(
                    pp[:, 1, :],
                    a_t[32:32 + R + 1, (k + 1) * 128:(k + 2) * 128],
                    bt_t[32:32 + R + 1, osl],
                    start=True, stop=True, tile_position=(32, 0),
                )
                mods[(og, p)] = pp

            def mul_pair(og, p):
                """One wide DVE multiply covering both chunks of pair p."""
                k = 2 * p
                osl = slice(og * 512, (og + 1) * 512)
                nc.vector.tensor_mul(
                    awt[:, k:k + 2, osl],
                    mods.pop((og, p))[:],
                    pend.pop((og, p))[:],
                )
                mul_done[(og, p)] = tick()
                mul_done[(og, k)] = mul_done[(og, k + 1)] = tick()

            def main_mm(po, m, og, k):
                assert (og, k) in mul_done, f"main ({m},{og},{k}) before mul"
                osl = slice(og * 512, (og + 1) * 512)
                nc.tensor.matmul(
                    po[:],
                    xbt[m][:, k, :],
                    awt[:, k, osl],
                    start=(k == 0),
                    stop=(k == KT - 1),
                )

            def drain(po, label, m, og):
                osl = slice(og * 512, (og + 1) * 512)
                ot = obp.tile([128, 512], f32, tag="ot")
                nc.vector.tensor_add(ot[:], po[:], bias_t[:, osl])
                nc.sync.dma_start(out=OUTP[m, :, osl], in_=ot[:])
                po_freed[label] = tick()

            # ---- prologue ----
            nc.sync.dma_start(out=a_t[:], in_=AAUG[:])
            nc.sync.dma_start(out=bt_t[:], in_=BTAUG[:])
            for p in range(2):
                wt_dma(0, p)
            load_xb(0)
            for p in range(2, 4):
                wt_dma(0, p)
            load_xb(1)

            nc.vector.memset(wu_l[:], 0.0)
            nc.vector.memset(wu_r[:], 0.0)
            for i in range(SCRATCH):
                pp = ppp.tile([128, 2, 512], f32, tag="pp", name=f"scr{i}")
                pp_alloc(("scr", i))
                nc.tensor.matmul(
                    pp[:, 0, :], wu_l[:], wu_r[:], start=True, stop=True
                )

            mod_pair(0, 0)

            # ---- phase B: og0 production + staggered og0 mains m=0..3;
            #      og1 production begins in the tail ----
            po0 = []
            for m in range(M0):
                po_alloc(("og0", m))
                po0.append(
                    opp.tile([128, 512], f32, tag="po", name=f"po0_{m}")
                )
            NJ_B = BSTART[-1] // 2 + KT // 2 + 1  # 30
            for j in range(NJ_B):
                if j == 0:
                    load_xb(2)
                if j == 1:
                    load_xb(3)
                if j == 2:
                    nc.sync.dma_start(out=bias_t[:], in_=BIASB[:])
                # og0 weight staging, 4 pairs ahead
                if j + 4 < KT // 2:
                    wt_dma(0, j + 4)
                # og0 mul for pair j
                if j < KT // 2:
                    mul_pair(0, j)
                # og0 mods one pair ahead
                if j + 1 < KT // 2:
                    mod_pair(0, j + 1)
                # PE densifier while only group 0 is active
                if j <= 4:
                    nc.tensor.matmul(
                        po0[3][:], wu_l[:], wu_r[:],
                        start=True, stop=True, skip_group_check=True,
                    )
                # ---- og1 production dripped into the tail ----
                if j == 15:
                    wt_dma(1, 0)
                if j == 16:
                    wt_dma(1, 1)
                    mod_pair(1, 0)
                if j == 17:
                    mul_pair(1, 0)
                    mod_pair(1, 1)
                if j == 18:
                    wt_dma(1, 2)
                    mul_pair(1, 1)
                    mod_pair(1, 2)
                # og0 mains, block j-1
                for m in range(M0):
                    for t in (0, 1):
                        k = 2 * (j - 1) + t - BSTART[m]
                        if 0 <= k < KT:
                            main_mm(po0[m], m, 0, k)
                # og0-m0 drain right after its stop (j == 17)
                if j == 17:
                    drain(po0[0], ("og0", 0), 0, 0)

            # ---- phase C: og1 mains m=0..3 + remaining og1 production ----
            po1 = {}
            for j in range(CS[-1] // 2 + KT // 2):  # 19
                if j == 4:
                    load_xb(4)
                if j == 10:
                    load_xb(5)
                # og1 weight staging, pairs ahead
                if j + 3 < KT // 2:
                    wt_dma(1, j + 3)
                # og1 mul one pair ahead of mains
                if j + 2 < KT // 2:
                    mul_pair(1, j + 2)
                # og1 mods two pairs ahead
                if j + 3 < KT // 2:
                    mod_pair(1, j + 3)
                # drains of og0 groups, one iter before og1 group m starts
                m = j + 1
                if 1 <= m < M0:
                    drain(po0[m], ("og0", m), m, 0)
                # og1 group starts: group m starts mains at j == CS[m] // 2
                for m in range(M0):
                    if j == CS[m] // 2:
                        po_alloc(("og1", m))
                        po1[m] = opp.tile(
                            [128, 512], f32, tag="po", name=f"po1_{m}"
                        )
                # og1 mains, block j
                for m in range(M0):
                    for t in (0, 1):
                        k = 2 * j + t - CS[m]
                        if 0 <= k < KT:
                            main_mm(po1[m], m, 1, k)

            # ---- phase D: m=4..31, both output halves per x tile ----
            for m in range(M0):
                drain(po1[m], ("og1", m), m, 1)
            for m in range(M0, MT):
                if m + 2 < MT:
                    load_xb(m + 2)
                for og in range(2):
                    if m == MT - 1 and og == 1:
                        # final group: two sequential 256-wide halves in
                        # separate full banks; half A's drain hides under
                        # half B's accumulation, shrinking the exposed tail
                        for h, (o0, o1) in enumerate(((512, 768), (768, 1024))):
                            po_alloc(("d", m, og, h))
                            po_freed[("d", m, og, h)] = tick()
                            ph = opp.tile(
                                [128, 512], f32, tag="po", name=f"ph{h}"
                            )
                            for k in range(KT):
                                nc.tensor.matmul(
                                    ph[:, 0:256],
                                    xbt[m][:, k, :],
                                    awt[:, k, o0:o1],
                                    start=(k == 0),
                                    stop=(k == KT - 1),
                                )
                            ot = obp.tile(
                                [128, 256], f32, tag="ot", name=f"oth{h}"
                            )
                            nc.vector.tensor_add(
                                ot[:], ph[:, 0:256], bias_t[:, o0:o1]
                            )
                            nc.sync.dma_start(
                                out=OUTP[m, :, o0:o1], in_=ot[:]
                            )
                        continue
                    po_alloc(("d", m, og))
                    po = opp.tile([128, 512], f32, tag="po")
                    for k in range(KT):
                        main_mm(po, m, og, k)
                    drain(po, ("d", m, og), m, og)

            assert not mods and not pend, (list(mods), list(pend))

    nc.compile()
    return nc


def _get_nc():
    global _NC_CACHE
    if _NC_CACHE is None:
        _NC_CACHE = _build_nc()
    return _NC_CACHE


def kernel(x, weight, bias, lora_A, lora_B):
    global LAST_RESULT
    from concourse.bass_utils import run_bass_kernel_spmd

    x = np.asarray(x, dtype=np.float32)
    weight = np.asarray(weight, dtype=np.float32)
    bias = np.asarray(bias, dtype=np.float32)
    lora_A = np.asarray(lora_A, dtype=np.float32)
    lora_B = np.asarray(lora_B, dtype=np.float32)

    x2 = x.reshape(TOK, IN)

    # x blocked per token-half: [m, p=i%128, k=i//128, t=tok%128] bf16
    xbs = []
    for tb in range(TB):
        xh = x2[tb * TOKH:(tb + 1) * TOKH]
        xb = xh.reshape(MT, 128, KT, 128).transpose(0, 3, 2, 1)  # [m,p,k,t]
        xbs.append(np.ascontiguousarray(xb.astype(BF16)))

    # lora consts replicated at partition offsets 0 and 32 (row-group pairs)
    a1 = np.concatenate([lora_A, np.ones((1, IN), np.float32)], axis=0)
    a_aug = np.zeros((RP, IN), np.float32)
    a_aug[0:R + 1] = a1
    a_aug[32:32 + R + 1] = a1
    a_aug = a_aug.astype(BF16)

    wt0s, wt1s, bts, biases = [], [], [], []
    for ob in range(OB):
        osl = slice(ob * OQ, (ob + 1) * OQ)
        wq = weight[osl]                                   # [OQ, IN]
        wt = wq.T.reshape(KT, 128, OQ).astype(BF16)        # [k, p, o] bf16
        # paired: [pair, p, t(chunk-in-pair), o-half]
        wt0s.append(np.ascontiguousarray(
            wt[:, :, 0:512].reshape(KT // 2, 2, 128, 512).transpose(0, 2, 1, 3)
        ))
        wt1s.append(np.ascontiguousarray(
            wt[:, :, 512:1024].reshape(KT // 2, 2, 128, 512).transpose(0, 2, 1, 3)
        ))
        bq = lora_B[osl]                                   # [OQ, R]
        b1 = np.concatenate([bq.T, np.ones((1, OQ), np.float32)], axis=0)
        bt = np.zeros((RP, OQ), np.float32)
        bt[0:R + 1] = b1
        bt[32:32 + R + 1] = b1
        bts.append(bt.astype(BF16))
        biases.append(np.ascontiguousarray(np.tile(bias[osl][None, :], (128, 1))))

    in_maps = []
    for c in range(N_CORES):
        tb, ob = c // OB, c % OB
        in_maps.append(
            {
                "xb": xbs[tb],
                "wt0": wt0s[ob],
                "wt1": wt1s[ob],
                "a_aug": a_aug,
                "bt_aug": bts[ob],
                "bias_b": biases[ob],
            }
        )

    nc = _get_nc()
    res = run_bass_kernel_spmd(
        nc, in_maps, core_ids=list(range(N_CORES)), trace=TRACE
    )
    LAST_RESULT = res

    # reassemble: out[c] is [MT, 128, OQ] -> [TOKH, OQ]
    halves = []
    for tb in range(TB):
        cols = [
            res.results[tb * OB + ob]["out"].reshape(TOKH, OQ)
            for ob in range(OB)
        ]
        halves.append(np.concatenate(cols, axis=1))
    full = np.concatenate(halves, axis=0).reshape(B, S, OUT)
    return full
